# revision 37
# baseline (speedup 1.0000x reference)
"""Trainium2 Bass kernel for nn_Attention_Layer (B=4, S=2048, D=1024, fp32).

Sharding: 8 cores = 4 batches x 2 KEY-halves (flash-attention style).
Each core computes K/V projections for its 1024-key half, Q for a
1024-query half, AllGathers Q across the pair (the gather's rank-order
concat IS the global query order, so every downstream access stays
SPMD-uniform), computes exp-scores against its key half and ships the
UNNORMALIZED attention output N = exp(S)@V plus the per-query partial
denominator Z. The host combines halves: y = (N0 + N1) / (Z0 + Z1).

Compute dtypes: projections run the PE in bf16 (x and W shipped bf16);
the scores matmul runs in fp8-e4m3 with MatmulPerfMode.DoubleRow (two
128-deep k-tiles contracted per instruction, 2x bf16 rate); exp on ACT
in fp32; attn-weights @ V in bf16. The K-projection bias is dropped:
score terms that depend only on the query row cancel in softmax.

Z rides along as column 1024 (all-ones) of the V tile: the attn @ V
matmuls per (qt, ko) are 384+384+257 wide, so every weight (re)load
hides behind a longer moving phase and Z needs no extra 1-wide matmuls.

DMA uses both hardware DGE queues: weights stream on the scalar queue,
x / bounce / outputs on the sync queue, so the first weight tile isn't
stuck behind the x stream. A short burst of dummy matmuls at kernel
start keeps the PE's HAM clock gate warm through the initial DMA wait.
"""

import numpy as np

import concourse.mybir as mybir
import concourse.tile as tile
from concourse import bacc
from concourse.bass_utils import run_bass_kernel_spmd

B, S, D = 4, 2048, 1024
P = 128
KH = S // 2              # keys per core
QH = S // 2              # queries projected per core (CC mode)
EO = D // P              # 8 e-tiles (feature dim outer)
DO = D // P              # 8 d-tiles (contraction outer)
KO = KH // P             # 8 key 128-tiles per core
QC = S // 512            # 4 query 512-chunks
KC = KH // 512           # 2 key 512-chunks
EP = EO // 2             # 4 eo-pairs for DoubleRow
QT = S // P              # 16 query 128-tiles
VW = D + 1               # V free width incl. the ones column for Z
SCALE = 1.0 / np.sqrt(D)

USE_CC = True            # AllGather the Q projection across core pairs

F32 = mybir.dt.float32
BF16 = mybir.dt.bfloat16
FP8 = mybir.dt.float8e4
DR = mybir.MatmulPerfMode.DoubleRow
ACTF = mybir.ActivationFunctionType


def build_nc():
    nc = bacc.Bacc("TRN2", target_bir_lowering=False)

    XW = S if not USE_CC else KH     # x columns actually needed on-core
    xT = nc.dram_tensor("xT", [D, XW], BF16, kind="ExternalInput")
    xT8 = nc.dram_tensor("xT8", [D, KH], FP8, kind="ExternalInput")
    Wk = nc.dram_tensor("Wk", [EO, P, DO * P], FP8, kind="ExternalInput")
    Wq = nc.dram_tensor("Wq", [EO, P, DO * P], BF16, kind="ExternalInput")
    Wv = nc.dram_tensor("Wv", [D, D], BF16, kind="ExternalInput")
    bqT = nc.dram_tensor("bqT", [P, EO], F32, kind="ExternalInput")
    bv = nc.dram_tensor("bv", [P, D], BF16, kind="ExternalInput")
    y = nc.dram_tensor("y", [S, D], BF16, kind="ExternalOutput")
    z = nc.dram_tensor("z", [P, QT], F32, kind="ExternalOutput")

    xTr = xT.ap().rearrange("(do p) s -> p do s", p=P)
    Wvr = Wv.ap().rearrange("(do p) e -> p do e", p=P)

    with tile.TileContext(nc) as tc:
        with (
            tc.tile_pool(name="xts", bufs=1) as xts_pool,
            tc.tile_pool(name="wk", bufs=1) as wk_pool,         # 16KB
            tc.tile_pool(name="wq", bufs=1) as wq_pool,         # 16KB
            tc.tile_pool(name="wve", bufs=1) as wve_pool,       # 16KB
            tc.tile_pool(name="k8", bufs=1) as k8_pool,         # 8KB
            tc.tile_pool(name="q8", bufs=1) as q8_pool,         # 16KB
            tc.tile_pool(name="qs8", bufs=1) as qs8_pool,       # 8KB staging
            tc.tile_pool(name="vt", bufs=1) as v_pool,
            tc.tile_pool(name="pt", bufs=1) as p_pool,          # 32KB
            tc.tile_pool(name="outp", bufs=8) as out_pool,
            tc.tile_pool(name="small", bufs=1) as small_pool,
            tc.tile_pool(name="ps", bufs=6, space="PSUM") as ps_pool,
            tc.tile_pool(name="warm", bufs=1, space="PSUM") as warm_pool,
            tc.tile_pool(name="dram", bufs=2, space="DRAM") as dram_pool,
        ):
            bq_sb = small_pool.tile([P, EO], F32, tag="bq")
            bv_sb = small_pool.tile([P, D], BF16, tag="bv")
            ones_sb = small_pool.tile([P, 1], BF16, tag="ones")
            zs_sb = small_pool.tile([P, QT], F32, tag="zs")

            nc.vector.memset(ones_sb[:], 1.0)

            # keep the PE busy (HAM warm) while the first x/W DMAs land
            warm_ps = warm_pool.tile([1, 8], F32, tag="warm")
            for _ in range(240):
                nc.tensor.matmul(
                    warm_ps[:, 0:1], ones_sb[:], ones_sb[:],
                    start=True, stop=True,
                )

            xts = xts_pool.tile([P, DO, XW], BF16, tag="xts")
            x8 = xts_pool.tile([P, DO, KH], FP8, tag="x8")
            wk_sb = wk_pool.tile([P, EO, DO, P], FP8, tag="wk")
            wq_sb = wq_pool.tile([P, EO, DO, P], BF16, tag="wq")
            wve = wve_pool.tile([P, DO, D], BF16, tag="wve")
            k8 = k8_pool.tile([P, EO, KH], FP8, tag="k8")
            q8 = q8_pool.tile([P, EO, S], FP8, tag="q8")
            v_sb = v_pool.tile([P, KO, VW], BF16, tag="vt")
            p_sb = p_pool.tile([P, KO, S], BF16, tag="pt")

            # ---- DMA issue: the measured-fast hardware DGE queue is the
            # ---- scalar one; stream everything the front of the kernel
            # ---- needs there, in consumption order. The sync queue only
            # ---- carries the CC readback and z. --------------------------
            def _wslice(Wt, eo):
                return Wt[eo].unsqueeze(0).rearrange(
                    "o p (do e) -> (o p) do e", do=DO
                )

            # Each dma_start instruction costs ~650ns on its issuing engine,
            # so the ACT (scalar) queue gets as FEW instructions as possible
            # in front of the Q-phase activations; everything the later
            # phases need streams from the otherwise-idle sync engine.
            x8r = xT8.ap().rearrange("(do p) s -> p do s", p=P)
            wqr = Wq.ap().rearrange("eo p (do e) -> p eo do e", do=DO)
            nc.scalar.dma_start(wq_sb[:, 0], _wslice(Wq, 0))
            nc.scalar.dma_start(xts[:, :, 0:512], xTr[:, :, 0:512])
            nc.scalar.dma_start(xts[:, :, 512:XW], xTr[:, :, 512:XW])
            nc.scalar.dma_start(bq_sb[:], bqT[:, :])
            nc.scalar.dma_start(bv_sb[:], bv[:, :])
            nc.scalar.dma_start(wq_sb[:, 1:4], wqr[:, 1:4])
            nc.scalar.dma_start(wq_sb[:, 4:EO], wqr[:, 4:EO])
            for eo in range(EO):
                nc.sync.dma_start(wk_sb[:, eo], _wslice(Wk, eo))
            for do in range(DO):
                nc.sync.dma_start(x8[:, do, :], x8r[:, do, :])
            for do in range(DO):
                nc.sync.dma_start(wve[:, do, :], Wvr[:, do, :])
            # ones column of V for the Z ride-along
            nc.vector.memset(v_sb[:, :, D : D + 1], 1.0)

            # ---- Q projection (own query half), bias via ACT Identity ----
            if USE_CC:
                q_stage = qs8_pool.tile([P, EO, QH], FP8, tag="qs8")
                for eo in range(EO):
                    for qc in range(QH // 512):
                        ps0 = ps_pool.tile([P, 512], F32, tag="ps")
                        for do in range(DO):
                            nc.tensor.matmul(
                                ps0[:], wq_sb[:, eo, do],
                                xts[:, do, qc * 512 : qc * 512 + 512],
                                start=(do == 0), stop=(do == DO - 1),
                            )
                        nc.scalar.activation(
                            q_stage[:, eo, qc * 512 : qc * 512 + 512], ps0[:],
                            ACTF.Identity, bias=bq_sb[:, eo : eo + 1],
                        )
                # pair-AllGather: out = [rank0 half, rank1 half] = global order
                cc_in = dram_pool.tile([P, EO, QH], FP8, tag="ccin")
                cc_out = dram_pool.tile([2, P, EO, QH], FP8, tag="ccout")
                nc.scalar.dma_start(cc_in[:], q_stage[:])
                nc.gpsimd.collective_compute(
                    "AllGather",
                    mybir.AluOpType.bypass,
                    replica_groups=[[0, 1], [2, 3], [4, 5], [6, 7]],
                    ins=[cc_in[:].opt()],
                    outs=[cc_out[:].opt()],
                )
                for r in range(2):
                    nc.sync.dma_start(
                        q8[:, :, r * QH : (r + 1) * QH],
                        cc_out[r],
                    )
            else:
                for eo in range(EO):
                    for qc in range(QC):
                        ps0 = ps_pool.tile([P, 512], F32, tag="ps")
                        for do in range(DO):
                            nc.tensor.matmul(
                                ps0[:], wq_sb[:, eo, do],
                                xts[:, do, qc * 512 : qc * 512 + 512],
                                start=(do == 0), stop=(do == DO - 1),
                            )
                        nc.scalar.activation(
                            q8[:, eo, qc * 512 : qc * 512 + 512], ps0[:],
                            ACTF.Identity, bias=bq_sb[:, eo : eo + 1],
                        )

            # ---- K projection (key half, no bias: cancels in softmax) ----
            # runs fully in fp8 DoubleRow: K only feeds the scores matmul,
            # which already quantizes to e4m3, so the extra input-side
            # noise is affordable (measured rel_l2 0.0112 -> ~0.016).
            for eo in range(EO):
                for kc in range(KC):
                    ps0 = ps_pool.tile([P, 512], F32, tag="ps")
                    for dp in range(DO // 2):
                        nc.tensor.matmul(
                            ps0[:], wk_sb[:, eo, 2 * dp : 2 * dp + 2],
                            x8[:, 2 * dp : 2 * dp + 2, kc * 512 : kc * 512 + 512],
                            start=(dp == 0), stop=(dp == DO // 2 - 1),
                            perf_mode=DR,
                        )
                    nc.scalar.activation(
                        k8[:, eo, kc * 512 : kc * 512 + 512], ps0[:], ACTF.Copy,
                    )

            # ---- V projection (key half) ---------------------------------
            for kt in range(KO):
                ps0 = ps_pool.tile([P, 512], F32, tag="ps")
                ps1 = ps_pool.tile([P, 512], F32, tag="ps")
                for do in range(DO):
                    xkt = xts[:, do, kt * P : (kt + 1) * P]
                    nc.tensor.matmul(
                        ps0[:], xkt, wve[:, do, 0:512],
                        start=(do == 0), stop=(do == DO - 1),
                    )
                    nc.tensor.matmul(
                        ps1[:], xkt, wve[:, do, 512:1024],
                        start=(do == 0), stop=(do == DO - 1),
                    )
                nc.vector.tensor_tensor(
                    v_sb[:, kt, 0:512], ps0[:], bv_sb[:, 0:512],
                    mybir.AluOpType.add,
                )
                nc.vector.tensor_tensor(
                    v_sb[:, kt, 512:1024], ps1[:], bv_sb[:, 512:1024],
                    mybir.AluOpType.add,
                )

            # ---- scores^T: exp(K.T @ Q / sqrt(D)), fp8 DoubleRow ---------
            for qc in range(QC):
                for kt in range(KO):
                    ps0 = ps_pool.tile([P, 512], F32, tag="ps")
                    for ep in range(EP):
                        nc.tensor.matmul(
                            ps0[:],
                            k8[:, 2 * ep : 2 * ep + 2, kt * P : (kt + 1) * P],
                            q8[:, 2 * ep : 2 * ep + 2, qc * 512 : qc * 512 + 512],
                            start=(ep == 0), stop=(ep == EP - 1),
                            perf_mode=DR,
                        )
                    nc.scalar.activation(
                        p_sb[:, kt, qc * 512 : qc * 512 + 512], ps0[:],
                        ACTF.Exp, scale=float(SCALE),
                    )

            # ---- attention numerator N = P^T.T @ [V | 1] -----------------
            # chunks 384/384/257: every ldweights hides behind the moving
            # phase of the previous matmul; Z is column 1024.
            for qt in range(QT):
                av0 = ps_pool.tile([P, 512], F32, tag="ps")
                av1 = ps_pool.tile([P, 512], F32, tag="ps")
                av2 = ps_pool.tile([P, 512], F32, tag="ps")
                for ko in range(KO):
                    lhs = p_sb[:, ko, qt * P : (qt + 1) * P]
                    nc.tensor.matmul(
                        av0[:, 0:384], lhs, v_sb[:, ko, 0:384],
                        start=(ko == 0), stop=(ko == KO - 1),
                    )
                    nc.tensor.matmul(
                        av1[:, 0:384], lhs, v_sb[:, ko, 384:768],
                        start=(ko == 0), stop=(ko == KO - 1),
                    )
                    nc.tensor.matmul(
                        av2[:, 0:257], lhs, v_sb[:, ko, 768 : 768 + 257],
                        start=(ko == 0), stop=(ko == KO - 1),
                    )
                oy = out_pool.tile([P, D], BF16, tag="oy")
                nc.scalar.activation(oy[:, 0:384], av0[:, 0:384], ACTF.Copy)
                nc.scalar.activation(oy[:, 384:768], av1[:, 0:384], ACTF.Copy)
                nc.scalar.activation(oy[:, 768:1024], av2[:, 0:256], ACTF.Copy)
                nc.scalar.activation(
                    zs_sb[:, qt : qt + 1], av2[:, 256:257], ACTF.Copy
                )
                # alternate output queues so one slow DGE can't stall av
                eng = nc.scalar if qt % 2 == 0 else nc.sync
                eng.dma_start(y[qt * P : (qt + 1) * P, :], oy[:])
            nc.sync.dma_start(z[:, :], zs_sb[:])

    nc.finalize()
    return nc


_NC_CACHE = None


def make_in_maps(x, Wk, bk, Wq, bq, Wv, bv):
    import ml_dtypes

    bf16 = ml_dtypes.bfloat16
    x = np.asarray(x, dtype=np.float32)

    e4m3 = ml_dtypes.float8_e4m3

    def _wre(W, dt):
        # [D, D] -> [EO, P(part), DO*P] so each e-tile slice is one
        # fully contiguous per-partition DMA
        W = np.asarray(W, np.float32).reshape(DO, P, EO, P)
        return np.ascontiguousarray(
            W.transpose(2, 1, 0, 3).reshape(EO, P, DO * P).astype(dt)
        )

    Wk8 = _wre(Wk, e4m3)
    Wq8 = _wre(Wq, bf16)
    Wv8 = np.ascontiguousarray(np.asarray(Wv, np.float32).astype(bf16))
    bqT = np.ascontiguousarray(np.asarray(bq, np.float32).reshape(EO, P).T)
    bv2 = np.ascontiguousarray(
        np.broadcast_to(
            np.asarray(bv, np.float32).reshape(1, D), (P, D)
        ).astype(bf16)
    )

    in_maps = []
    for c in range(8):
        b, kh = c // 2, c % 2
        xTb = x[b].T.astype(bf16)                          # [D, S]
        if USE_CC:
            # core's keys AND its projected query half are cols [kh*KH,...)
            xTb = np.ascontiguousarray(xTb[:, kh * KH : (kh + 1) * KH])
        elif kh == 1:
            # swap the s-halves so the key half is always cols [0, KH)
            xTb = np.ascontiguousarray(
                np.concatenate([xTb[:, KH:], xTb[:, :KH]], axis=1)
            )
        else:
            xTb = np.ascontiguousarray(xTb)
        in_maps.append(
            {
                "xT": xTb,
                "xT8": np.ascontiguousarray(xTb[:, 0:KH].astype(e4m3)),
                "Wk": Wk8, "Wq": Wq8, "Wv": Wv8,
                "bqT": bqT, "bv": bv2,
            }
        )
    return in_maps


def gather_out(results):
    out = np.empty((B, S, D), dtype=np.float32)
    for b in range(B):
        r0, r1 = results[2 * b], results[2 * b + 1]
        n0 = np.asarray(r0["y"], dtype=np.float32)          # [S, D]
        n1 = np.asarray(r1["y"], dtype=np.float32)
        z0 = np.asarray(r0["z"], dtype=np.float32).T.reshape(S)
        z1 = np.asarray(r1["z"], dtype=np.float32).T.reshape(S)
        if not USE_CC:
            # core 1 saw queries in swapped-half order; map back to global
            n1 = np.concatenate([n1[KH:], n1[:KH]], axis=0)
            z1 = np.concatenate([z1[KH:], z1[:KH]], axis=0)
        out[b] = (n0 + n1) / (z0 + z1)[:, None]
    return out


def kernel(x, Wk, bk, Wq, bq, Wv, bv):
    global _NC_CACHE
    if _NC_CACHE is None:
        _NC_CACHE = build_nc()
    in_maps = make_in_maps(x, Wk, bk, Wq, bq, Wv, bv)
    res = run_bass_kernel_spmd(_NC_CACHE, in_maps, list(range(8)))
    return gather_out(res.results)
